# revision 42
# baseline (speedup 1.0000x reference)
"""GPT2 causal attention (B=2, T=2048, C=1024, H=16) on 8 TRN2 NeuronCores.

Sharding: core g = (batch b = g//4, head-group hg = g%4 of 4 heads).
Tensor-parallel over heads (column-split W_attn, row-split W_proj) x
data-parallel over batch. Each core computes a full [T, C] partial of the
output projection for its 4 heads; host sums the 4 partials per batch and
adds b_proj. No collectives.

Per-core kernel (bf16 matmuls, fp32 PSUM), cost-model-aware layout:
  Matmul time is charged per MOVING column only, so every matmul keeps its
  moving operand as small as possible:
    scores  S^T[tk, tq] = kT_tile^T @ qT          (moving tq, causal-trimmed)
    AV      yu[tq, 65]  = expS^T_tile^T @ V_aug   (moving 65 = 64 v + ones)
  The ones column of V_aug makes yu[:, 64] the softmax row-sum, which lands
  per-PARTITION, so normalization is a [128,1] DVE reciprocal + per-partition
  tensor_scalar multiply - no cross-partition broadcast needed. Normalized
  [tq, d] head-pair tiles are transposed to yT[d, tq] by the DMA xbar
  (dma_start_transpose), keeping the PE free for real matmuls. QKV runs in
  tq-512 column rounds so compute starts ~2us into the x DMA, and leftover
  QKV/proj matmuls are interleaved into the attention stream as filler to
  keep the tensor engine continuously busy (p-state) while the scalar engine
  exp's the scores.
"""

import numpy as np
import ml_dtypes
from collections import deque

BF16 = ml_dtypes.bfloat16

B, T, C, H, D = 2, 2048, 1024, 16, 64
HL = 4          # heads per core
DL = HL * D     # 256 local head dims
N_CORES = 8
NT = T // 128   # 16 tk tiles
NJ = T // 512   # 4 tq groups
SCALE = 1.0 / np.sqrt(D)
VW = HL * 65    # V row stride per tk-tile (per head: 64 data + 1 ones col)

FILL_NS = 900   # PE filler budget per attention pair-step

_CACHE = {}


def _build_program():
    import concourse.tile as tile
    from concourse import bacc
    import concourse.mybir as mybir

    f32 = mybir.dt.float32
    bf16 = mybir.dt.bfloat16
    Exp = mybir.ActivationFunctionType.Exp

    nc = bacc.Bacc("TRN2", target_bir_lowering=False, debug=False)

    # ---- DRAM I/O (host pre-sharded and pre-packed to SBUF layout) ----
    xT_d = nc.dram_tensor("xTp", [128, 8 * T], bf16, kind="ExternalInput").ap()
    wq_d = nc.dram_tensor("wqp", [128, 8 * DL], bf16, kind="ExternalInput").ap()
    wk_d = nc.dram_tensor("wkp", [128, 8 * DL], bf16, kind="ExternalInput").ap()
    wv_d = nc.dram_tensor("wvp", [128, 8 * DL], bf16, kind="ExternalInput").ap()
    wp_d = nc.dram_tensor("wpp", [128, 2 * C], bf16, kind="ExternalInput").ap()
    bq_d = nc.dram_tensor("bq", [128, 2], f32, kind="ExternalInput").ap()
    bk_d = nc.dram_tensor("bk", [128, 2], f32, kind="ExternalInput").ap()
    bvr_d = nc.dram_tensor("bvr", [128, DL], f32, kind="ExternalInput").ap()
    tri_d = nc.dram_tensor("tri", [128, 128], bf16, kind="ExternalInput").ap()
    ident_d = nc.dram_tensor("ident", [128, 128], bf16, kind="ExternalInput").ap()
    # two half-contraction partials (heads 0-1 / heads 2-3); host sums them
    out_d = nc.dram_tensor("out", [2, T, C], bf16, kind="ExternalOutput").ap()

    with tile.TileContext(nc) as tc:
        with (
            tc.tile_pool(name="const", bufs=1) as cpool,
            tc.tile_pool(name="exp", bufs=6) as epool,
            tc.tile_pool(name="small", bufs=8) as spool,
            tc.tile_pool(name="ostage", bufs=3) as opool,
            tc.tile_pool(name="psbig", bufs=2, space="PSUM") as pbig,
            tc.tile_pool(name="pssc", bufs=2, space="PSUM") as pscp,
            tc.tile_pool(name="psyu", bufs=2, space="PSUM") as pyup,
        ):
            # ---- persistent SBUF ----
            xT = cpool.tile([128, 8 * T], bf16, tag="xT")       # c-chunk c at [:, c*T:]
            wq = cpool.tile([128, 8 * DL], bf16, tag="wq")
            wk = cpool.tile([128, 8 * DL], bf16, tag="wk")
            wv = cpool.tile([128, 8 * DL], bf16, tag="wv")
            wp = cpool.tile([128, 2 * C], bf16, tag="wp")       # d-chunk dc at [:, dc*C:]
            bq = cpool.tile([128, 2], f32, tag="bq")
            bk = cpool.tile([128, 2], f32, tag="bk")
            bvr = cpool.tile([128, DL], f32, tag="bvr")
            tri = cpool.tile([128, 128], bf16, tag="tri")
            ident = cpool.tile([128, 128], bf16, tag="ident")
            qT = cpool.tile([128, 2 * T], bf16, tag="qT")       # head h: [64*(h%2):, (h//2)*T + t]
            kT = cpool.tile([128, 2 * T], bf16, tag="kT")
            yT = cpool.tile([128, 2 * T], bf16, tag="yT")
            V = cpool.tile([128, NT * VW], bf16, tag="V")       # tk-tile tt, head h at [:, tt*VW + 65*h : +65]

            # ---- input DMAs ----
            # HWDGE descriptor-gen costs ~625ns per DMA instruction, so the
            # startup path uses as few instructions as possible: wq halves,
            # then x(tb0) per-chunk (these gate the first q/k rounds), then
            # everything else as whole-tensor DMAs. Small side inputs issue
            # from the Activation engine's DGE queue off the SP critical path.
            nc.sync.dma_start(out=wq[:, 0:4 * DL], in_=wq_d[:, 0:4 * DL])
            for c in range(4):
                s = slice(c * T, c * T + 512)
                nc.sync.dma_start(out=xT[:, s], in_=xT_d[:, s])
            nc.sync.dma_start(out=wq[:, 4 * DL:], in_=wq_d[:, 4 * DL:])
            for c in range(4, 8):
                s = slice(c * T, c * T + 512)
                nc.sync.dma_start(out=xT[:, s], in_=xT_d[:, s])
            nc.sync.dma_start(out=wk[:, :], in_=wk_d[:, :])
            nc.scalar.dma_start(out=bq[:, :], in_=bq_d[:, :])
            nc.scalar.dma_start(out=bk[:, :], in_=bk_d[:, :])
            nc.scalar.dma_start(out=tri[:, :], in_=tri_d[:, :])
            nc.sync.dma_start(out=wv[:, :], in_=wv_d[:, :])
            nc.scalar.dma_start(out=bvr[:, :], in_=bvr_d[:, :])
            nc.scalar.dma_start(out=ident[:, :], in_=ident_d[:, :])
            for c in range(8):  # tb1..3 in one contiguous span per chunk
                s = slice(c * T + 512, (c + 1) * T)
                nc.sync.dma_start(out=xT[:, s], in_=xT_d[:, s])
            nc.sync.dma_start(out=wp[:, :], in_=wp_d[:, :])

            # ones columns for the row-sum trick; data cols overwritten below
            nc.gpsimd.memset(V[:, :], 1.0)

            # ---- emission closures ----
            uid = iter(range(1 << 20))

            def qk_parts(w_sb, b_sb, dst, dc, tb, lbl):
                """Two closures: c=0..3 accumulate, c=4..7 + bias eviction."""
                st = {}

                def part1():
                    ps = pbig.tile([128, 512], f32, tag="big",
                                   name=f"qk_{lbl}_{next(uid)}")
                    st["ps"] = ps
                    for c in range(4):
                        nc.tensor.matmul(
                            ps[:, :],
                            w_sb[:, c * DL + dc * 128: c * DL + (dc + 1) * 128],
                            xT[:, c * T + tb * 512: c * T + (tb + 1) * 512],
                            start=(c == 0), stop=False,
                        )

                def part2():
                    ps = st["ps"]
                    for c in range(4, 8):
                        nc.tensor.matmul(
                            ps[:, :],
                            w_sb[:, c * DL + dc * 128: c * DL + (dc + 1) * 128],
                            xT[:, c * T + tb * 512: c * T + (tb + 1) * 512],
                            start=False, stop=(c == 7),
                        )
                    nc.vector.tensor_scalar_add(
                        dst[:, dc * T + tb * 512: dc * T + (tb + 1) * 512],
                        ps[:, :], b_sb[:, dc:dc + 1],
                    )

                return [(lbl + "p1", 880, part1), (lbl + "p2", 1030, part2)]

            def v_parts(tt):
                st = {}

                def part1():
                    ps = pbig.tile([128, DL], f32, tag="big",
                                   name=f"v_{tt}_{next(uid)}")
                    st["ps"] = ps
                    for c in range(4):
                        nc.tensor.matmul(
                            ps[:, :],
                            xT[:, c * T + tt * 128: c * T + (tt + 1) * 128],
                            wv[:, c * DL:(c + 1) * DL],
                            start=(c == 0), stop=False,
                        )

                def part2():
                    ps = st["ps"]
                    for c in range(4, 8):
                        nc.tensor.matmul(
                            ps[:, :],
                            xT[:, c * T + tt * 128: c * T + (tt + 1) * 128],
                            wv[:, c * DL:(c + 1) * DL],
                            start=False, stop=(c == 7),
                        )
                    vdst = V[:, tt * VW:(tt + 1) * VW].rearrange(
                        "p (h e) -> p h e", h=HL)[:, :, 0:64]
                    nc.vector.tensor_add(
                        vdst,
                        ps[:, :].rearrange("p (h e) -> p h e", h=HL),
                        bvr[:, :].rearrange("p (h e) -> p h e", h=HL),
                    )
                return [(f"v{tt}a", 440, part1), (f"v{tt}", 560, part2)]

            def make_proj(tt, dc, tail=False):
                def go():
                    ot = opool.tile([128, C], bf16, tag="ot",
                                    name=f"ot_{tt}_{dc}_{next(uid)}")
                    for cc in range(2):
                        pp = pbig.tile([128, 512], f32, tag="big",
                                       name=f"pp_{tt}_{dc}_{cc}_{next(uid)}")
                        nc.tensor.matmul(
                            pp[:, :],
                            yT[:, dc * T + tt * 128: dc * T + (tt + 1) * 128],
                            wp[:, dc * C + cc * 512: dc * C + (cc + 1) * 512],
                            start=True, stop=True,
                        )
                        if tail and cc == 1:
                            nc.scalar.copy(ot[:, cc * 512:(cc + 1) * 512],
                                           pp[:, :])
                        else:
                            nc.vector.tensor_copy(
                                ot[:, cc * 512:(cc + 1) * 512], pp[:, :])
                        if tail:
                            # tail: per-half DMAs on alternating queues start
                            # draining before the second half is computed
                            eng = nc.scalar if (tt + cc) % 2 else nc.sync
                            eng.dma_start(
                                out=out_d[dc, tt * 128:(tt + 1) * 128,
                                          cc * 512:(cc + 1) * 512],
                                in_=ot[:, cc * 512:(cc + 1) * 512])
                    if not tail:
                        nc.sync.dma_start(
                            out=out_d[dc, tt * 128:(tt + 1) * 128, :],
                            in_=ot[:, :])
                return (f"proj{tt}d{dc}", 900, go)

            # ---- fill queue: opportunistic PE work to hide exp latency ----
            # Balance-based: track estimated ns emitted on Act (exp stream,
            # the pacer) vs PE, and pop filler whenever PE falls behind.
            fillq = deque()
            done = set()
            est = {"act": 0.0, "pe": 0.0}

            def fill(budget=None):
                budget = FILL_NS if budget is None else budget
                while fillq and budget > 0:
                    lbl, cost, fn = fillq.popleft()
                    fn()
                    done.add(lbl)
                    budget -= cost

            def drain_until(lbl):
                if lbl in done:
                    return
                while fillq:
                    l, cost, fn = fillq.popleft()
                    fn()
                    done.add(l)
                    est["pe"] += cost
                    if l == lbl:
                        return

            # ---- attention block for head-pair hp, tq-group j (512 wide) ----
            def attn(hp, j, prediag=None):
                fb = hp * T
                ni = 4 * (j + 1)
                np_ = ni // 2
                yu = [pyup.tile([128, HL * 65], f32, tag="yu",
                                name=f"yu_{hp}_{j}_{he}") for he in (0, 1)]

                def emit_av(p, ets_p):
                    """AV matmuls for i-pair p (lagged one pair behind exp).
                    One PSUM accumulation group spans the whole yu bank:
                    start's zero-region clear covers all four sub-blocks,
                    whose bytes zero lazily on first write, so only the
                    first/last matmul into the tile carry start/stop."""
                    i0, i1 = 2 * p, 2 * p + 1
                    for he in (0, 1):
                        h = 2 * hp + he
                        et = ets_p[he]
                        for idx, ii in enumerate((i0, i1)):
                            if hp == 0:
                                drain_until(f"v{ii}")
                            for q in range(max(0, ii - 4 * j), 4):
                                nc.tensor.matmul(
                                    yu[he][:, q * 65:(q + 1) * 65],
                                    et[:, idx * 512 + q * 128: idx * 512 + (q + 1) * 128],
                                    V[:, ii * VW + 65 * h: ii * VW + 65 * h + 65],
                                    start=(ii == 0 and q == 0),
                                    stop=(ii == 4 * j + 3 and q == 3),
                                )
                            est["pe"] += (4 - max(0, ii - 4 * j)) * 65 * 0.42

                prev_ets = None
                for p in range(np_):
                    if p == 2 * j and prediag is not None:
                        prediag()   # k tiles for the diagonal pairs land here
                    i0, i1 = 2 * p, 2 * p + 1
                    d0 = max(0, 128 * (i0 - 4 * j))   # even-tile causal trim
                    q0e = i0 - 4 * j                  # diag sub-block (if >=0)
                    q0o = i1 - 4 * j
                    ets = []
                    for he in (0, 1):
                        po = 64 * he
                        sc = pscp.tile([128, 1024], f32, tag="sc",
                                       name=f"sc_{hp}_{j}_{p}_{he}")
                        for idx, ii in enumerate((i0, i1)):
                            nc.tensor.matmul(
                                sc[:, idx * 512 + d0:(idx + 1) * 512],
                                kT[po:po + 64, fb + ii * 128: fb + (ii + 1) * 128],
                                qT[po:po + 64, fb + j * 512 + d0: fb + (j + 1) * 512],
                                start=True, stop=True,
                            )
                        et = epool.tile([128, 1024], bf16, tag="exp",
                                        name=f"et_{hp}_{j}_{p}_{he}")
                        if d0 == 0:
                            nc.scalar.activation(et[:, :], sc[:, :], Exp,
                                                 scale=float(SCALE))
                        else:
                            et3 = et[:, :].rearrange("p (g q) -> p g q", g=2)
                            sc3 = sc[:, :].rearrange("p (g q) -> p g q", g=2)
                            nc.scalar.activation(et3[:, :, d0:512],
                                                 sc3[:, :, d0:512], Exp,
                                                 scale=float(SCALE))
                        est["pe"] += 2 * (512 - d0) * 0.42
                        est["act"] += 2 * (512 - d0) * 0.833 + 250
                        # mask the 128x128 diagonal blocks
                        if q0e >= 0:
                            sl = slice(q0e * 128, (q0e + 1) * 128)
                            nc.vector.tensor_mul(et[:, sl], et[:, sl], tri[:, :])
                        if 0 <= q0o < 4:
                            sl = slice(512 + q0o * 128, 512 + (q0o + 1) * 128)
                            nc.vector.tensor_mul(et[:, sl], et[:, sl], tri[:, :])
                        ets.append(et)
                    if prev_ets is not None:
                        emit_av(p - 1, prev_ets)
                    fill()
                    prev_ets = ets
                fill()
                emit_av(np_ - 1, prev_ets)
                # evictions: all reads must follow the bank's group stop
                rcps = []
                for he in (0, 1):
                    rcp = spool.tile([128, 4], f32, tag="rcp",
                                     name=f"rcp_{hp}_{j}_{he}")
                    nc.vector.reciprocal(
                        rcp[:, :],
                        yu[he][:, :].rearrange("p (q e) -> p q e", q=4)[:, :, 64])
                    rcps.append(rcp)
                for q in range(4):
                    t = 4 * j + q
                    ybn = spool.tile([128, 128], bf16, tag="ybn",
                                     name=f"ybn_{hp}_{t}")
                    for he in (0, 1):
                        if hp == 1 and j == 3:
                            # tail: Act is idle once the last exp retires
                            nc.scalar.mul(ybn[:, he * 64:(he + 1) * 64],
                                          yu[he][:, q * 65: q * 65 + 64],
                                          rcps[he][:, q:q + 1])
                        else:
                            nc.vector.tensor_scalar_mul(
                                ybn[:, he * 64:(he + 1) * 64],
                                yu[he][:, q * 65: q * 65 + 64],
                                rcps[he][:, q:q + 1])
                    if hp == 1 and j == 3:
                        # tail block: a PE transpose + DVE copy is ~3x lower
                        # latency than the DMA xbar path
                        tp = pbig.tile([128, 128], bf16, tag="big",
                                       name=f"tp_{t}", padded_shape=[128, 512])
                        nc.tensor.transpose(tp[:, :], ybn[:, :], ident[:, :])
                        nc.vector.tensor_copy(
                            yT[:, fb + t * 128: fb + (t + 1) * 128], tp[:, :])
                        est["pe"] += 55
                    else:
                        nc.sync.dma_start_transpose(
                            yT[:, fb + t * 128: fb + (t + 1) * 128], ybn[:, :])
                    fillq.append(make_proj(t, hp, tail=(hp == 1 and j == 3)))

            # ---- schedule ----
            # prefix: q/k over tq [0:512) for all four heads; the dc1 rounds
            # only need wq/wk + x(tb0), and bridge the wk/wv DMA latency.
            for lbl, cost, fn in qk_parts(wq, bq, qT, 0, 0, "qdc0tb0"):
                fn()
            for lbl, cost, fn in qk_parts(wq, bq, qT, 1, 0, "qdc1tb0"):
                fn()
            for lbl, cost, fn in qk_parts(wk, bk, kT, 0, 0, "kdc0tb0"):
                fn()
            for lbl, cost, fn in qk_parts(wk, bk, kT, 1, 0, "kdc1tb0"):
                fn()
            done.add("qdc1tb0p2")
            done.add("kdc1tb0p2")

            # fill queue: V tiles as needed by hp0's AV wavefront, then the
            # dc1 (heads 2,3) q/k rounds consumed during hp0's attention.
            for tt in range(8):
                fillq.extend(v_parts(tt))
            for tt in range(8, 12):
                fillq.extend(v_parts(tt))
            fillq.extend(qk_parts(wq, bq, qT, 1, 1, "qdc1tb1"))
            fillq.extend(qk_parts(wk, bk, kT, 1, 1, "kdc1tb1"))
            for tt in range(12, 16):
                fillq.extend(v_parts(tt))
            fillq.extend(qk_parts(wq, bq, qT, 1, 2, "qdc1tb2"))
            fillq.extend(qk_parts(wk, bk, kT, 1, 2, "kdc1tb2"))
            fillq.extend(qk_parts(wq, bq, qT, 1, 3, "qdc1tb3"))
            fillq.extend(qk_parts(wk, bk, kT, 1, 3, "kdc1tb3"))

            # hp0: q(tb j) before block j; k(tb j) deferred to the diagonal
            def run_inline(parts):
                def go():
                    for lbl, cost, fn in parts:
                        fn()
                        est["pe"] += cost
                        done.add(lbl)
                return go

            attn(0, 0)
            for j in (1, 2, 3):
                run_inline(qk_parts(wq, bq, qT, 0, j, f"qdc0tb{j}"))()
                attn(0, j,
                     prediag=run_inline(qk_parts(wk, bk, kT, 0, j,
                                                 f"kdc0tb{j}")))

            # hp1: q/k rounds come from the fill queue; proj unlocks per tile
            for j in range(4):
                drain_until(f"qdc1tb{j}p2")
                attn(1, j,
                     prediag=(lambda jj=j: drain_until(f"kdc1tb{jj}p2")))

            # tail: any remaining filler (last proj tiles)
            while fillq:
                lbl, cost, fn = fillq.popleft()
                fn()
                done.add(lbl)

    nc.compile()
    return nc


def get_program():
    if "nc" not in _CACHE:
        _CACHE["nc"] = _build_program()
    return _CACHE["nc"]


def _pack_cmajor(a):
    """[C_rows, N] -> [128, (C_rows/128)*N] with chunk c at [:, c*N:(c+1)*N]."""
    rows, n = a.shape
    return np.ascontiguousarray(
        a.reshape(rows // 128, 128, n).transpose(1, 0, 2).reshape(128, -1))


def make_in_maps(x, W_attn, b_attn, W_proj):
    """Host-side sharding: per-core input dict."""
    x = np.asarray(x, np.float32)
    W_attn = np.asarray(W_attn, np.float32)
    b_attn = np.asarray(b_attn, np.float32)
    W_proj = np.asarray(W_proj, np.float32)

    tk = np.arange(128)[:, None]
    tq = np.arange(128)[None, :]
    tri = (tq >= tk).astype(BF16)
    ident = np.eye(128, dtype=BF16)

    xT_b = [_pack_cmajor(x[b].T.astype(BF16)) for b in range(B)]

    in_maps = []
    for g in range(N_CORES):
        b, hg = divmod(g, 4)
        cs = slice(hg * DL, (hg + 1) * DL)
        wq = _pack_cmajor(W_attn[:, 0 * C:1 * C][:, cs].astype(BF16))
        wk = _pack_cmajor(W_attn[:, 1 * C:2 * C][:, cs].astype(BF16))
        wv = _pack_cmajor(W_attn[:, 2 * C:3 * C][:, cs].astype(BF16))
        wp = _pack_cmajor(W_proj[cs, :].astype(BF16))
        bq = np.ascontiguousarray(b_attn[0 * C:1 * C][cs].reshape(2, 128).T)
        bk = np.ascontiguousarray(b_attn[1 * C:2 * C][cs].reshape(2, 128).T)
        bvr = np.ascontiguousarray(np.tile(b_attn[2 * C:3 * C][cs][None, :], (128, 1)))
        in_maps.append({
            "xTp": xT_b[b],
            "wqp": wq, "wkp": wk, "wvp": wv, "wpp": wp,
            "bq": bq.astype(np.float32), "bk": bk.astype(np.float32),
            "bvr": bvr.astype(np.float32),
            "tri": tri, "ident": ident,
        })
    return in_maps


def assemble_output(results, b_proj):
    """results: per-core dicts with 'out' [T, C] partials."""
    b_proj = np.asarray(b_proj, np.float32)
    out = np.zeros((B, T, C), np.float32)
    for g in range(N_CORES):
        o = np.asarray(results[g]["out"], np.float32)
        out[g // 4] += o[0] + o[1]
    out += b_proj[None, None, :]
    return out


def kernel(x, W_attn, b_attn, W_proj, b_proj):
    from concourse.bass_utils import run_bass_kernel_spmd

    nc = get_program()
    in_maps = make_in_maps(x, W_attn, b_attn, W_proj)
    res = run_bass_kernel_spmd(nc, in_maps, list(range(N_CORES)))
    return assemble_output(res.results, b_proj)


# revision 48
# speedup vs baseline: 1.1316x; 1.1316x over previous
"""GPT2 causal attention (B=2, T=2048, C=1024, H=16) on 8 TRN2 NeuronCores.

Sharding: core g = (batch b = g//4, head-group hg = g%4 of 4 heads).
Tensor-parallel over heads (column-split W_attn, row-split W_proj) x
data-parallel over batch. Each core computes a full [T, C] partial of the
output projection for its 4 heads; host sums the 4 partials per batch and
adds b_proj. No collectives.

Per-core kernel (bf16 matmuls, fp32 PSUM), cost-model-aware layout:
  Matmul time is charged per MOVING column only, so every matmul keeps its
  moving operand as small as possible:
    scores  S^T[tk, tq] = kT_tile^T @ qT          (moving tq, causal-trimmed)
    AV      yu[tq, 65]  = expS^T_tile^T @ V_aug   (moving 65 = 64 v + ones)
  The ones column of V_aug makes yu[:, 64] the softmax row-sum, which lands
  per-PARTITION, so normalization is a [128,1] DVE reciprocal + per-partition
  tensor_scalar multiply - no cross-partition broadcast needed. Normalized
  [tq, d] head-pair tiles are transposed to yT[d, tq] by the DMA xbar
  (dma_start_transpose), keeping the PE free for real matmuls. QKV runs in
  tq-512 column rounds so compute starts ~2us into the x DMA, and leftover
  QKV/proj matmuls are interleaved into the attention stream as filler to
  keep the tensor engine continuously busy (p-state) while the scalar engine
  exp's the scores.
"""

import numpy as np
import ml_dtypes
from collections import deque

BF16 = ml_dtypes.bfloat16

B, T, C, H, D = 2, 2048, 1024, 16, 64
HL = 4          # heads per core
DL = HL * D     # 256 local head dims
N_CORES = 8
NT = T // 128   # 16 tk tiles
NJ = T // 512   # 4 tq groups
SCALE = 1.0 / np.sqrt(D)
VW = HL * 65    # V row stride per tk-tile (per head: 64 data + 1 ones col)

FILL_NS = 900   # PE filler budget per attention pair-step
TAIL_J = 0      # hp1 block processed last (shortest tail chain)

_CACHE = {}


def _build_program():
    import concourse.tile as tile
    from concourse import bacc
    import concourse.mybir as mybir

    f32 = mybir.dt.float32
    bf16 = mybir.dt.bfloat16
    Exp = mybir.ActivationFunctionType.Exp

    nc = bacc.Bacc("TRN2", target_bir_lowering=False, debug=False)

    # ---- DRAM I/O (host pre-sharded and pre-packed to SBUF layout) ----
    xT_d = nc.dram_tensor("xTp", [128, 8 * T], bf16, kind="ExternalInput").ap()
    wq_d = nc.dram_tensor("wqp", [128, 8 * DL], bf16, kind="ExternalInput").ap()
    wk_d = nc.dram_tensor("wkp", [128, 8 * DL], bf16, kind="ExternalInput").ap()
    wv_d = nc.dram_tensor("wvp", [128, 8 * DL], bf16, kind="ExternalInput").ap()
    wp_d = nc.dram_tensor("wpp", [128, 2 * C], bf16, kind="ExternalInput").ap()
    bq_d = nc.dram_tensor("bq", [128, 2], f32, kind="ExternalInput").ap()
    bk_d = nc.dram_tensor("bk", [128, 2], f32, kind="ExternalInput").ap()
    bvr_d = nc.dram_tensor("bvr", [128, DL], f32, kind="ExternalInput").ap()
    tri_d = nc.dram_tensor("tri", [128, 128], bf16, kind="ExternalInput").ap()
    ident_d = nc.dram_tensor("ident", [128, 128], bf16, kind="ExternalInput").ap()
    out_d = nc.dram_tensor("out", [T, C], bf16, kind="ExternalOutput").ap()

    with tile.TileContext(nc) as tc:
        with (
            tc.tile_pool(name="const", bufs=1) as cpool,
            tc.tile_pool(name="exp", bufs=6) as epool,
            tc.tile_pool(name="small", bufs=8) as spool,
            tc.tile_pool(name="ostage", bufs=3) as opool,
            tc.tile_pool(name="psbig", bufs=2, space="PSUM") as pbig,
            tc.tile_pool(name="pssc", bufs=2, space="PSUM") as pscp,
            tc.tile_pool(name="psyu", bufs=2, space="PSUM") as pyup,
        ):
            # ---- persistent SBUF ----
            xT = cpool.tile([128, 8 * T], bf16, tag="xT")       # c-chunk c at [:, c*T:]
            wq = cpool.tile([128, 8 * DL], bf16, tag="wq")
            wk = cpool.tile([128, 8 * DL], bf16, tag="wk")
            wv = cpool.tile([128, 8 * DL], bf16, tag="wv")
            wp = cpool.tile([128, 2 * C], bf16, tag="wp")       # d-chunk dc at [:, dc*C:]
            bq = cpool.tile([128, 2], f32, tag="bq")
            bk = cpool.tile([128, 2], f32, tag="bk")
            bvr = cpool.tile([128, DL], f32, tag="bvr")
            tri = cpool.tile([128, 128], bf16, tag="tri")
            ident = cpool.tile([128, 128], bf16, tag="ident")
            qT = cpool.tile([128, 2 * T], bf16, tag="qT")       # head h: [64*(h%2):, (h//2)*T + t]
            kT = cpool.tile([128, 2 * T], bf16, tag="kT")
            yT = cpool.tile([128, 2 * T], bf16, tag="yT")
            V = cpool.tile([128, NT * VW], bf16, tag="V")       # tk-tile tt, head h at [:, tt*VW + 65*h : +65]

            # ---- input DMAs ----
            # HWDGE descriptor-gen costs ~625ns per DMA instruction, so the
            # startup path uses as few instructions as possible: wq halves,
            # then x(tb0) per-chunk (these gate the first q/k rounds), then
            # everything else as whole-tensor DMAs. Small side inputs issue
            # from the Activation engine's DGE queue off the SP critical path.
            nc.sync.dma_start(out=wq[:, 0:4 * DL], in_=wq_d[:, 0:4 * DL])
            for c in range(4):
                s = slice(c * T, c * T + 512)
                nc.sync.dma_start(out=xT[:, s], in_=xT_d[:, s])
            nc.sync.dma_start(out=wq[:, 4 * DL:], in_=wq_d[:, 4 * DL:])
            for c in range(4, 8):
                s = slice(c * T, c * T + 512)
                nc.sync.dma_start(out=xT[:, s], in_=xT_d[:, s])
            nc.sync.dma_start(out=wk[:, :], in_=wk_d[:, :])
            nc.scalar.dma_start(out=bq[:, :], in_=bq_d[:, :])
            nc.scalar.dma_start(out=bk[:, :], in_=bk_d[:, :])
            nc.scalar.dma_start(out=tri[:, :], in_=tri_d[:, :])
            nc.sync.dma_start(out=wv[:, :], in_=wv_d[:, :])
            nc.scalar.dma_start(out=bvr[:, :], in_=bvr_d[:, :])
            nc.scalar.dma_start(out=ident[:, :], in_=ident_d[:, :])
            for c in range(8):  # tb1..3 in one contiguous span per chunk
                s = slice(c * T + 512, (c + 1) * T)
                nc.sync.dma_start(out=xT[:, s], in_=xT_d[:, s])
            nc.sync.dma_start(out=wp[:, :], in_=wp_d[:, :])

            # ones columns for the row-sum trick; data cols overwritten below
            nc.gpsimd.memset(V[:, :], 1.0)

            # ---- emission closures ----
            uid = iter(range(1 << 20))

            def qk_parts(w_sb, b_sb, dst, dc, tb, lbl):
                """Two closures: c=0..3 accumulate, c=4..7 + bias eviction."""
                st = {}

                def part1():
                    ps = pbig.tile([128, 512], f32, tag="big",
                                   name=f"qk_{lbl}_{next(uid)}")
                    st["ps"] = ps
                    for c in range(4):
                        nc.tensor.matmul(
                            ps[:, :],
                            w_sb[:, c * DL + dc * 128: c * DL + (dc + 1) * 128],
                            xT[:, c * T + tb * 512: c * T + (tb + 1) * 512],
                            start=(c == 0), stop=False,
                        )

                def part2():
                    ps = st["ps"]
                    for c in range(4, 8):
                        nc.tensor.matmul(
                            ps[:, :],
                            w_sb[:, c * DL + dc * 128: c * DL + (dc + 1) * 128],
                            xT[:, c * T + tb * 512: c * T + (tb + 1) * 512],
                            start=False, stop=(c == 7),
                        )
                    nc.vector.tensor_scalar_add(
                        dst[:, dc * T + tb * 512: dc * T + (tb + 1) * 512],
                        ps[:, :], b_sb[:, dc:dc + 1],
                    )

                return [(lbl + "p1", 880, part1), (lbl + "p2", 1030, part2)]

            def v_parts(tt):
                st = {}

                def part1():
                    ps = pbig.tile([128, DL], f32, tag="big",
                                   name=f"v_{tt}_{next(uid)}")
                    st["ps"] = ps
                    for c in range(4):
                        nc.tensor.matmul(
                            ps[:, :],
                            xT[:, c * T + tt * 128: c * T + (tt + 1) * 128],
                            wv[:, c * DL:(c + 1) * DL],
                            start=(c == 0), stop=False,
                        )

                def part2():
                    ps = st["ps"]
                    for c in range(4, 8):
                        nc.tensor.matmul(
                            ps[:, :],
                            xT[:, c * T + tt * 128: c * T + (tt + 1) * 128],
                            wv[:, c * DL:(c + 1) * DL],
                            start=False, stop=(c == 7),
                        )
                    vdst = V[:, tt * VW:(tt + 1) * VW].rearrange(
                        "p (h e) -> p h e", h=HL)[:, :, 0:64]
                    nc.vector.tensor_add(
                        vdst,
                        ps[:, :].rearrange("p (h e) -> p h e", h=HL),
                        bvr[:, :].rearrange("p (h e) -> p h e", h=HL),
                    )
                return [(f"v{tt}a", 440, part1), (f"v{tt}", 560, part2)]

            def make_proj(tt, tail=False):
                def go():
                    ot = opool.tile([128, C], bf16, tag="ot",
                                    name=f"ot_{tt}_{next(uid)}")
                    for cc in range(2):
                        pp = pbig.tile([128, 512], f32, tag="big",
                                       name=f"pp_{tt}_{cc}_{next(uid)}")
                        for dc in range(2):
                            nc.tensor.matmul(
                                pp[:, :],
                                yT[:, dc * T + tt * 128: dc * T + (tt + 1) * 128],
                                wp[:, dc * C + cc * 512: dc * C + (cc + 1) * 512],
                                start=(dc == 0), stop=(dc == 1),
                            )
                        if tail and cc == 1:
                            nc.scalar.copy(ot[:, cc * 512:(cc + 1) * 512],
                                           pp[:, :])
                        else:
                            nc.vector.tensor_copy(
                                ot[:, cc * 512:(cc + 1) * 512], pp[:, :])
                        if tail:
                            # tail: per-half DMAs on alternating queues start
                            # draining before the second half is computed
                            eng = nc.scalar if (tt + cc) % 2 else nc.sync
                            eng.dma_start(
                                out=out_d[tt * 128:(tt + 1) * 128,
                                          cc * 512:(cc + 1) * 512],
                                in_=ot[:, cc * 512:(cc + 1) * 512])
                    if not tail:
                        nc.sync.dma_start(
                            out=out_d[tt * 128:(tt + 1) * 128, :], in_=ot[:, :])
                return (f"proj{tt}", 1040, go)

            # ---- fill queue: opportunistic PE work to hide exp latency ----
            # Balance-based: track estimated ns emitted on Act (exp stream,
            # the pacer) vs PE, and pop filler whenever PE falls behind.
            fillq = deque()
            done = set()
            est = {"act": 0.0, "pe": 0.0}

            def fill(budget=None):
                budget = FILL_NS if budget is None else budget
                while fillq and budget > 0:
                    lbl, cost, fn = fillq.popleft()
                    fn()
                    done.add(lbl)
                    budget -= cost

            def drain_until(lbl):
                if lbl in done:
                    return
                while fillq:
                    l, cost, fn = fillq.popleft()
                    fn()
                    done.add(l)
                    est["pe"] += cost
                    if l == lbl:
                        return

            # ---- attention block for head-pair hp, tq-group j (512 wide) ----
            def attn(hp, j, prediag=None):
                fb = hp * T
                ni = 4 * (j + 1)
                np_ = ni // 2
                yu = [pyup.tile([128, HL * 65], f32, tag="yu",
                                name=f"yu_{hp}_{j}_{he}") for he in (0, 1)]

                def emit_av(p, ets_p):
                    """AV matmuls for i-pair p (lagged one pair behind exp).
                    One PSUM accumulation group spans the whole yu bank:
                    start's zero-region clear covers all four sub-blocks,
                    whose bytes zero lazily on first write, so only the
                    first/last matmul into the tile carry start/stop."""
                    i0, i1 = 2 * p, 2 * p + 1
                    for he in (0, 1):
                        h = 2 * hp + he
                        et = ets_p[he]
                        for idx, ii in enumerate((i0, i1)):
                            if hp == 0:
                                drain_until(f"v{ii}")
                            for q in range(max(0, ii - 4 * j), 4):
                                nc.tensor.matmul(
                                    yu[he][:, q * 65:(q + 1) * 65],
                                    et[:, idx * 512 + q * 128: idx * 512 + (q + 1) * 128],
                                    V[:, ii * VW + 65 * h: ii * VW + 65 * h + 65],
                                    start=(ii == 0 and q == 0),
                                    stop=(ii == 4 * j + 3 and q == 3),
                                )
                            est["pe"] += (4 - max(0, ii - 4 * j)) * 65 * 0.42

                prev_ets = None
                for p in range(np_):
                    if p == 2 * j and prediag is not None:
                        prediag()   # k tiles for the diagonal pairs land here
                    i0, i1 = 2 * p, 2 * p + 1
                    d0 = max(0, 128 * (i0 - 4 * j))   # even-tile causal trim
                    q0e = i0 - 4 * j                  # diag sub-block (if >=0)
                    q0o = i1 - 4 * j
                    ets = []
                    for he in (0, 1):
                        po = 64 * he
                        sc = pscp.tile([128, 1024], f32, tag="sc",
                                       name=f"sc_{hp}_{j}_{p}_{he}")
                        for idx, ii in enumerate((i0, i1)):
                            nc.tensor.matmul(
                                sc[:, idx * 512 + d0:(idx + 1) * 512],
                                kT[po:po + 64, fb + ii * 128: fb + (ii + 1) * 128],
                                qT[po:po + 64, fb + j * 512 + d0: fb + (j + 1) * 512],
                                start=True, stop=True,
                            )
                        et = epool.tile([128, 1024], bf16, tag="exp",
                                        name=f"et_{hp}_{j}_{p}_{he}")
                        if d0 == 0:
                            nc.scalar.activation(et[:, :], sc[:, :], Exp,
                                                 scale=float(SCALE))
                        else:
                            et3 = et[:, :].rearrange("p (g q) -> p g q", g=2)
                            sc3 = sc[:, :].rearrange("p (g q) -> p g q", g=2)
                            nc.scalar.activation(et3[:, :, d0:512],
                                                 sc3[:, :, d0:512], Exp,
                                                 scale=float(SCALE))
                        est["pe"] += 2 * (512 - d0) * 0.42
                        est["act"] += 2 * (512 - d0) * 0.833 + 250
                        # mask the 128x128 diagonal blocks
                        if q0e >= 0:
                            sl = slice(q0e * 128, (q0e + 1) * 128)
                            nc.vector.tensor_mul(et[:, sl], et[:, sl], tri[:, :])
                        if 0 <= q0o < 4:
                            sl = slice(512 + q0o * 128, 512 + (q0o + 1) * 128)
                            nc.vector.tensor_mul(et[:, sl], et[:, sl], tri[:, :])
                        ets.append(et)
                    if prev_ets is not None:
                        emit_av(p - 1, prev_ets)
                    fill()
                    prev_ets = ets
                fill()
                emit_av(np_ - 1, prev_ets)
                # evictions: all reads must follow the bank's group stop
                rcps = []
                for he in (0, 1):
                    rcp = spool.tile([128, 4], f32, tag="rcp",
                                     name=f"rcp_{hp}_{j}_{he}")
                    nc.vector.reciprocal(
                        rcp[:, :],
                        yu[he][:, :].rearrange("p (q e) -> p q e", q=4)[:, :, 64])
                    rcps.append(rcp)
                for q in range(4):
                    t = 4 * j + q
                    ybn = spool.tile([128, 128], bf16, tag="ybn",
                                     name=f"ybn_{hp}_{t}")
                    for he in (0, 1):
                        if hp == 1 and j == TAIL_J:
                            # tail: Act is idle once the last exp retires
                            nc.scalar.mul(ybn[:, he * 64:(he + 1) * 64],
                                          yu[he][:, q * 65: q * 65 + 64],
                                          rcps[he][:, q:q + 1])
                        else:
                            nc.vector.tensor_scalar_mul(
                                ybn[:, he * 64:(he + 1) * 64],
                                yu[he][:, q * 65: q * 65 + 64],
                                rcps[he][:, q:q + 1])
                    if hp == 1 and j == TAIL_J:
                        # tail block: a PE transpose + DVE copy is ~3x lower
                        # latency than the DMA xbar path
                        tp = pbig.tile([128, 128], bf16, tag="big",
                                       name=f"tp_{t}", padded_shape=[128, 512])
                        nc.tensor.transpose(tp[:, :], ybn[:, :], ident[:, :])
                        nc.vector.tensor_copy(
                            yT[:, fb + t * 128: fb + (t + 1) * 128], tp[:, :])
                    else:
                        nc.sync.dma_start_transpose(
                            yT[:, fb + t * 128: fb + (t + 1) * 128], ybn[:, :])
                    if hp == 1:
                        fillq.append(make_proj(t, tail=(j == TAIL_J)))

            # ---- schedule ----
            # prefix: q/k over tq [0:512) for all four heads; the dc1 rounds
            # only need wq/wk + x(tb0), and bridge the wk/wv DMA latency.
            for lbl, cost, fn in qk_parts(wq, bq, qT, 0, 0, "qdc0tb0"):
                fn()
            for lbl, cost, fn in qk_parts(wq, bq, qT, 1, 0, "qdc1tb0"):
                fn()
            for lbl, cost, fn in qk_parts(wk, bk, kT, 0, 0, "kdc0tb0"):
                fn()
            for lbl, cost, fn in qk_parts(wk, bk, kT, 1, 0, "kdc1tb0"):
                fn()
            done.add("qdc1tb0p2")
            done.add("kdc1tb0p2")

            # fill queue: V tiles as needed by hp0's AV wavefront, then the
            # dc1 (heads 2,3) q/k rounds consumed during hp0's attention.
            for tt in range(8):
                fillq.extend(v_parts(tt))
            for tt in range(8, 12):
                fillq.extend(v_parts(tt))
            fillq.extend(qk_parts(wq, bq, qT, 1, 1, "qdc1tb1"))
            fillq.extend(qk_parts(wk, bk, kT, 1, 1, "kdc1tb1"))
            for tt in range(12, 16):
                fillq.extend(v_parts(tt))
            fillq.extend(qk_parts(wq, bq, qT, 1, 2, "qdc1tb2"))
            fillq.extend(qk_parts(wk, bk, kT, 1, 2, "kdc1tb2"))
            fillq.extend(qk_parts(wq, bq, qT, 1, 3, "qdc1tb3"))
            fillq.extend(qk_parts(wk, bk, kT, 1, 3, "kdc1tb3"))

            # hp0: q(tb j) before block j; k(tb j) deferred to the diagonal
            def run_inline(parts):
                def go():
                    for lbl, cost, fn in parts:
                        fn()
                        est["pe"] += cost
                        done.add(lbl)
                return go

            attn(0, 0)
            for j in (1, 2, 3):
                run_inline(qk_parts(wq, bq, qT, 0, j, f"qdc0tb{j}"))()
                attn(0, j,
                     prediag=run_inline(qk_parts(wk, bk, kT, 0, j,
                                                 f"kdc0tb{j}")))

            # hp1 in order [1, 2, 3, 0]: each block's proj tiles become the
            # next block's filler, and the tail block (j0) is the shortest
            # dependency chain with j3's proj tiles as its filler.
            for j in (1, 2, 3, 0):
                drain_until(f"qdc1tb{j}p2")
                attn(1, j,
                     prediag=(lambda jj=j: drain_until(f"kdc1tb{jj}p2")))

            # tail: any remaining filler (last proj tiles)
            while fillq:
                lbl, cost, fn = fillq.popleft()
                fn()
                done.add(lbl)

    nc.compile()
    return nc


def get_program():
    if "nc" not in _CACHE:
        _CACHE["nc"] = _build_program()
    return _CACHE["nc"]


def _pack_cmajor(a):
    """[C_rows, N] -> [128, (C_rows/128)*N] with chunk c at [:, c*N:(c+1)*N]."""
    rows, n = a.shape
    return np.ascontiguousarray(
        a.reshape(rows // 128, 128, n).transpose(1, 0, 2).reshape(128, -1))


def make_in_maps(x, W_attn, b_attn, W_proj):
    """Host-side sharding: per-core input dict."""
    x = np.asarray(x, np.float32)
    W_attn = np.asarray(W_attn, np.float32)
    b_attn = np.asarray(b_attn, np.float32)
    W_proj = np.asarray(W_proj, np.float32)

    tk = np.arange(128)[:, None]
    tq = np.arange(128)[None, :]
    tri = (tq >= tk).astype(BF16)
    ident = np.eye(128, dtype=BF16)

    xT_b = [_pack_cmajor(x[b].T.astype(BF16)) for b in range(B)]

    in_maps = []
    for g in range(N_CORES):
        b, hg = divmod(g, 4)
        cs = slice(hg * DL, (hg + 1) * DL)
        wq = _pack_cmajor(W_attn[:, 0 * C:1 * C][:, cs].astype(BF16))
        wk = _pack_cmajor(W_attn[:, 1 * C:2 * C][:, cs].astype(BF16))
        wv = _pack_cmajor(W_attn[:, 2 * C:3 * C][:, cs].astype(BF16))
        wp = _pack_cmajor(W_proj[cs, :].astype(BF16))
        bq = np.ascontiguousarray(b_attn[0 * C:1 * C][cs].reshape(2, 128).T)
        bk = np.ascontiguousarray(b_attn[1 * C:2 * C][cs].reshape(2, 128).T)
        bvr = np.ascontiguousarray(np.tile(b_attn[2 * C:3 * C][cs][None, :], (128, 1)))
        in_maps.append({
            "xTp": xT_b[b],
            "wqp": wq, "wkp": wk, "wvp": wv, "wpp": wp,
            "bq": bq.astype(np.float32), "bk": bk.astype(np.float32),
            "bvr": bvr.astype(np.float32),
            "tri": tri, "ident": ident,
        })
    return in_maps


def assemble_output(results, b_proj):
    """results: per-core dicts with 'out' [T, C] partials."""
    b_proj = np.asarray(b_proj, np.float32)
    out = np.zeros((B, T, C), np.float32)
    for g in range(N_CORES):
        o = np.asarray(results[g]["out"], np.float32)
        out[g // 4] += o[0] + o[1]
    out += b_proj[None, None, :]
    return out


def kernel(x, W_attn, b_attn, W_proj, b_proj):
    from concourse.bass_utils import run_bass_kernel_spmd

    nc = get_program()
    in_maps = make_in_maps(x, W_attn, b_attn, W_proj)
    res = run_bass_kernel_spmd(nc, in_maps, list(range(N_CORES)))
    return assemble_output(res.results, b_proj)


# revision 82
# speedup vs baseline: 1.2814x; 1.1324x over previous
"""GPT2 causal attention (B=2, T=2048, C=1024, H=16) on 8 TRN2 NeuronCores.

Sharding: core g = (batch b = g//4, head-group hg = g%4 of 4 heads).
Tensor-parallel over heads (column-split W_attn, row-split W_proj) x
data-parallel over batch. Each core computes a full [T, C] partial of the
output projection for its 4 heads; host sums the 4 partials per batch and
adds b_proj. No collectives.

Per-core kernel (bf16 matmuls, fp32 PSUM), cost-model-aware layout:
  PE matmul time is charged per MOVING column, so every matmul keeps its
  moving operand as small as possible:
    scores  S^T[tk, tq] = kT_tile^T @ qT          (moving tq, causal-trimmed)
    AV      yu[tq, 65]  = expS^T_tile^T @ V_aug   (moving 65 = 64 v + ones)
  The ones column of V_aug makes yu[:, 64] the softmax row-sum, which lands
  per-PARTITION, so normalization is a [128,1] DVE reciprocal + per-partition
  tensor_scalar multiply - no cross-partition broadcast needed. All four
  tq-sub-block accumulations share one PSUM bank as a single accumulation
  group (the start matmul's 2KB zero-region clear covers them; untouched
  bytes zero lazily on first write). Normalized [tq, d] head-pair tiles are
  transposed to yT[d, tq] by the DMA xbar (dma_start_transpose), keeping the
  PE free for real matmuls; the final (tail) block uses PE transposes + Act
  copies instead to shorten the drain chain.

  Schedule: x is DMA'd in tq-512 column rounds (few, large DMAs - HWDGE
  descriptor-gen is ~625ns per DMA instruction) so QKV starts ~3us in; all
  non-attention PE work (remaining q/k rounds, V tiles, output-projection
  tiles) lives in an ordered fill queue that is drained one closure per
  attention pair-step, keeping the tensor engine continuously busy (p-state)
  while the scalar engine streams the exps; drain_until markers force what
  each block needs before it runs. AV matmuls lag their exp by one pair-step
  so they never head-block the PE queue. hp1 runs j in [1,2,3,0] so each
  block's proj tiles fill the next block, and the short j0 block plus
  Act-assisted eviction forms the tail.
"""

import numpy as np
import ml_dtypes
from collections import deque

BF16 = ml_dtypes.bfloat16

B, T, C, H, D = 2, 2048, 1024, 16, 64
HL = 4          # heads per core
DL = HL * D     # 256 local head dims
N_CORES = 8
NT = T // 128   # 16 tk tiles
NJ = T // 512   # 4 tq groups
SCALE = 1.0 / np.sqrt(D)
VW = HL * 65    # V row stride per tk-tile (per head: 64 data + 1 ones col)

FILL_NS = 900   # PE filler budget per attention pair-step
TAIL_J = 0      # hp1 block processed last (shortest tail chain)
TAIL_FILL_NS = 2500  # wider filler budget inside the tail block

_CACHE = {}


def _build_program():
    import concourse.tile as tile
    from concourse import bacc
    import concourse.mybir as mybir

    f32 = mybir.dt.float32
    bf16 = mybir.dt.bfloat16
    Exp = mybir.ActivationFunctionType.Exp

    nc = bacc.Bacc("TRN2", target_bir_lowering=False, debug=False)

    # ---- DRAM I/O (host pre-sharded and pre-packed to SBUF layout) ----
    xT_d = nc.dram_tensor("xTp", [128, 8 * T], bf16, kind="ExternalInput").ap()
    wq_d = nc.dram_tensor("wqp", [128, 8 * DL], bf16, kind="ExternalInput").ap()
    wk_d = nc.dram_tensor("wkp", [128, 8 * DL], bf16, kind="ExternalInput").ap()
    wv_d = nc.dram_tensor("wvp", [128, 8 * DL], bf16, kind="ExternalInput").ap()
    wp_d = nc.dram_tensor("wpp", [128, 2 * C], bf16, kind="ExternalInput").ap()
    bq_d = nc.dram_tensor("bq", [128, 2], f32, kind="ExternalInput").ap()
    bk_d = nc.dram_tensor("bk", [128, 2], f32, kind="ExternalInput").ap()
    bvr_d = nc.dram_tensor("bvr", [128, DL], f32, kind="ExternalInput").ap()
    tri_d = nc.dram_tensor("tri", [128, 128], bf16, kind="ExternalInput").ap()
    ident_d = nc.dram_tensor("ident", [128, 128], bf16, kind="ExternalInput").ap()
    out_d = nc.dram_tensor("out", [T, C], bf16, kind="ExternalOutput").ap()

    with tile.TileContext(nc) as tc:
        with (
            tc.tile_pool(name="const", bufs=1) as cpool,
            tc.tile_pool(name="exp", bufs=6) as epool,
            tc.tile_pool(name="small", bufs=8) as spool,
            tc.tile_pool(name="ostage", bufs=3) as opool,
            tc.tile_pool(name="psbig", bufs=2, space="PSUM") as pbig,
            tc.tile_pool(name="pssc", bufs=2, space="PSUM") as pscp,
            tc.tile_pool(name="psyu", bufs=2, space="PSUM") as pyup,
        ):
            # ---- persistent SBUF ----
            xT = cpool.tile([128, 8 * T], bf16, tag="xT")       # c-chunk c at [:, c*T:]
            wq = cpool.tile([128, 8 * DL], bf16, tag="wq")
            wk = cpool.tile([128, 8 * DL], bf16, tag="wk")
            wv = cpool.tile([128, 8 * DL], bf16, tag="wv")
            wp = cpool.tile([128, 2 * C], bf16, tag="wp")       # d-chunk dc at [:, dc*C:]
            bq = cpool.tile([128, 2], f32, tag="bq")
            bk = cpool.tile([128, 2], f32, tag="bk")
            bvr = cpool.tile([128, DL], f32, tag="bvr")
            tri = cpool.tile([128, 128], bf16, tag="tri")
            ident = cpool.tile([128, 128], bf16, tag="ident")
            qT = cpool.tile([128, 2 * T], bf16, tag="qT")       # head h: [64*(h%2):, (h//2)*T + t]
            kT = cpool.tile([128, 2 * T], bf16, tag="kT")
            yT = cpool.tile([128, 2 * T], bf16, tag="yT")
            V = cpool.tile([128, NT * VW], bf16, tag="V")       # tk-tile tt, head h at [:, tt*VW + 65*h : +65]

            # ---- input DMAs ----
            # HWDGE descriptor-gen costs ~625ns per DMA instruction and the
            # SP/Act issue queues ~650ns each, so the startup path minimizes
            # instructions and spreads them over three queues: x(tb0) chunks
            # alternate SP/Act (these gate the first q/k rounds), while the
            # q/k weights and small side inputs go down the otherwise-idle
            # gpsimd SWDGE path, off both HWDGE queues entirely.
            nc.gpsimd.dma_start(out=wq[:, 0:4 * DL], in_=wq_d[:, 0:4 * DL])
            nc.gpsimd.dma_start(out=wk[:, 0:4 * DL], in_=wk_d[:, 0:4 * DL])
            nc.gpsimd.dma_start(out=wq[:, 4 * DL:], in_=wq_d[:, 4 * DL:])
            nc.gpsimd.dma_start(out=wk[:, 4 * DL:], in_=wk_d[:, 4 * DL:])
            nc.gpsimd.dma_start(out=bq[:, :], in_=bq_d[:, :])
            nc.gpsimd.dma_start(out=bk[:, :], in_=bk_d[:, :])
            nc.gpsimd.dma_start(out=tri[:, :], in_=tri_d[:, :])
            nc.gpsimd.dma_start(out=bvr[:, :], in_=bvr_d[:, :])
            nc.gpsimd.dma_start(out=ident[:, :], in_=ident_d[:, :])
            nc.sync.dma_start(out=wq[:, 0:4 * DL], in_=wq_d[:, 0:4 * DL])
            for c in range(4):
                s = slice(c * T, c * T + 512)
                nc.sync.dma_start(out=xT[:, s], in_=xT_d[:, s])
            nc.sync.dma_start(out=wq[:, 4 * DL:], in_=wq_d[:, 4 * DL:])
            for c in range(4, 8):
                s = slice(c * T, c * T + 512)
                nc.sync.dma_start(out=xT[:, s], in_=xT_d[:, s])
            nc.sync.dma_start(out=wk[:, :], in_=wk_d[:, :])
            nc.sync.dma_start(out=bq[:, :], in_=bq_d[:, :])
            nc.sync.dma_start(out=bk[:, :], in_=bk_d[:, :])
            nc.sync.dma_start(out=tri[:, :], in_=tri_d[:, :])
            nc.sync.dma_start(out=wv[:, :], in_=wv_d[:, :])
            nc.sync.dma_start(out=bvr[:, :], in_=bvr_d[:, :])
            nc.sync.dma_start(out=ident[:, :], in_=ident_d[:, :])
            for c in range(8):  # tb1..3 in one contiguous span per chunk
                s = slice(c * T + 512, (c + 1) * T)
                nc.sync.dma_start(out=xT[:, s], in_=xT_d[:, s])
            nc.sync.dma_start(out=wp[:, :], in_=wp_d[:, :])

            # ones columns for the row-sum trick; data cols overwritten below
            nc.gpsimd.memset(V[:, :], 1.0)

            # ---- emission closures ----
            uid = iter(range(1 << 20))

            def qk_parts(w_sb, b_sb, dst, dc, tb, lbl):
                """Two closures: c=0..3 accumulate, c=4..7 + bias eviction."""
                st = {}

                def part1():
                    ps = pbig.tile([128, 512], f32, tag="big",
                                   name=f"qk_{lbl}_{next(uid)}")
                    st["ps"] = ps
                    for c in range(4):
                        nc.tensor.matmul(
                            ps[:, :],
                            w_sb[:, c * DL + dc * 128: c * DL + (dc + 1) * 128],
                            xT[:, c * T + tb * 512: c * T + (tb + 1) * 512],
                            start=(c == 0), stop=False,
                        )

                def part2():
                    ps = st["ps"]
                    for c in range(4, 8):
                        nc.tensor.matmul(
                            ps[:, :],
                            w_sb[:, c * DL + dc * 128: c * DL + (dc + 1) * 128],
                            xT[:, c * T + tb * 512: c * T + (tb + 1) * 512],
                            start=False, stop=(c == 7),
                        )
                    nc.vector.tensor_scalar_add(
                        dst[:, dc * T + tb * 512: dc * T + (tb + 1) * 512],
                        ps[:, :], b_sb[:, dc:dc + 1],
                    )

                return [(lbl + "p1", 880, part1), (lbl + "p2", 1030, part2)]

            def v_parts(tt):
                st = {}

                def part1():
                    ps = pbig.tile([128, DL], f32, tag="big",
                                   name=f"v_{tt}_{next(uid)}")
                    st["ps"] = ps
                    for c in range(4):
                        nc.tensor.matmul(
                            ps[:, :],
                            xT[:, c * T + tt * 128: c * T + (tt + 1) * 128],
                            wv[:, c * DL:(c + 1) * DL],
                            start=(c == 0), stop=False,
                        )

                def part2():
                    ps = st["ps"]
                    for c in range(4, 8):
                        nc.tensor.matmul(
                            ps[:, :],
                            xT[:, c * T + tt * 128: c * T + (tt + 1) * 128],
                            wv[:, c * DL:(c + 1) * DL],
                            start=False, stop=(c == 7),
                        )
                    vdst = V[:, tt * VW:(tt + 1) * VW].rearrange(
                        "p (h e) -> p h e", h=HL)[:, :, 0:64]
                    nc.vector.tensor_add(
                        vdst,
                        ps[:, :].rearrange("p (h e) -> p h e", h=HL),
                        bvr[:, :].rearrange("p (h e) -> p h e", h=HL),
                    )
                return [(f"v{tt}a", 440, part1), (f"v{tt}", 560, part2)]

            def make_proj(tt, tail=False, steal=False):
                def go():
                    ot = opool.tile([128, C], bf16, tag="ot",
                                    name=f"ot_{tt}_{next(uid)}")
                    for cc in range(2):
                        # in the post-attention drain the sc banks are free;
                        # borrowing one doubles the pp pipeline depth
                        pool, ptag = ((pscp, "sc") if (steal and cc == 1)
                                      else (pbig, "big"))
                        pp = pool.tile([128, 512], f32, tag=ptag,
                                       name=f"pp_{tt}_{cc}_{next(uid)}")
                        for dc in range(2):
                            nc.tensor.matmul(
                                pp[:, :],
                                yT[:, dc * T + tt * 128: dc * T + (tt + 1) * 128],
                                wp[:, dc * C + cc * 512: dc * C + (cc + 1) * 512],
                                start=(dc == 0), stop=(dc == 1),
                            )
                        nc.vector.tensor_copy(
                            ot[:, cc * 512:(cc + 1) * 512], pp[:, :])
                        if tail:
                            nc.sync.dma_start(
                                out=out_d[tt * 128:(tt + 1) * 128,
                                          cc * 512:(cc + 1) * 512],
                                in_=ot[:, cc * 512:(cc + 1) * 512])
                    if not tail:
                        nc.sync.dma_start(
                            out=out_d[tt * 128:(tt + 1) * 128, :], in_=ot[:, :])
                return (f"proj{tt}", 1040, go)

            # ---- fill queue: opportunistic PE work to hide exp latency ----
            # Balance-based: track estimated ns emitted on Act (exp stream,
            # the pacer) vs PE, and pop filler whenever PE falls behind.
            fillq = deque()
            done = set()
            est = {"act": 0.0, "pe": 0.0}

            projq = deque()   # unlocked proj tiles, materialized lazily
            in_tail = [False]  # set while emitting the final attention block

            def fill(budget=None):
                budget = FILL_NS if budget is None else budget
                while budget > 0:
                    if fillq:
                        lbl, cost, fn = fillq.popleft()
                    elif projq:
                        lbl, cost, fn = make_proj(projq.popleft(),
                                                  tail=in_tail[0])
                    else:
                        return
                    fn()
                    done.add(lbl)
                    budget -= cost

            def drain_until(lbl):
                if lbl in done:
                    return
                while fillq:
                    l, cost, fn = fillq.popleft()
                    fn()
                    done.add(l)
                    est["pe"] += cost
                    if l == lbl:
                        return

            # ---- attention block for head-pair hp, tq-group j (512 wide) ----
            def attn(hp, j, prediag=None):
                fb = hp * T
                ni = 4 * (j + 1)
                np_ = ni // 2
                yu = [pyup.tile([128, HL * 65], f32, tag="yu",
                                name=f"yu_{hp}_{j}_{he}") for he in (0, 1)]

                def emit_av(p, ets_p):
                    """AV matmuls for i-pair p (lagged one pair behind exp).
                    One PSUM accumulation group spans the whole yu bank:
                    start's zero-region clear covers all four sub-blocks,
                    whose bytes zero lazily on first write, so only the
                    first/last matmul into the tile carry start/stop."""
                    i0, i1 = 2 * p, 2 * p + 1
                    for he in (0, 1):
                        h = 2 * hp + he
                        et = ets_p[he]
                        for idx, ii in enumerate((i0, i1)):
                            if hp == 0:
                                drain_until(f"v{ii}")
                            for q in range(max(0, ii - 4 * j), 4):
                                nc.tensor.matmul(
                                    yu[he][:, q * 65:(q + 1) * 65],
                                    et[:, idx * 512 + q * 128: idx * 512 + (q + 1) * 128],
                                    V[:, ii * VW + 65 * h: ii * VW + 65 * h + 65],
                                    start=(ii == 0 and q == 0),
                                    stop=(ii == 4 * j + 3 and q == 3),
                                )
                            est["pe"] += (4 - max(0, ii - 4 * j)) * 65 * 0.42

                etq = deque()
                for p in range(np_):
                    if p == 2 * j and prediag is not None:
                        prediag()   # k tiles for the diagonal pairs land here
                    i0, i1 = 2 * p, 2 * p + 1
                    d0 = max(0, 128 * (i0 - 4 * j))   # even-tile causal trim
                    q0e = i0 - 4 * j                  # diag sub-block (if >=0)
                    q0o = i1 - 4 * j
                    ets = []
                    for he in (0, 1):
                        po = 64 * he
                        sc = pscp.tile([128, 1024], f32, tag="sc",
                                       name=f"sc_{hp}_{j}_{p}_{he}")
                        for idx, ii in enumerate((i0, i1)):
                            nc.tensor.matmul(
                                sc[:, idx * 512 + d0:(idx + 1) * 512],
                                kT[po:po + 64, fb + ii * 128: fb + (ii + 1) * 128],
                                qT[po:po + 64, fb + j * 512 + d0: fb + (j + 1) * 512],
                                start=True, stop=True,
                            )
                        et = epool.tile([128, 1024], bf16, tag="exp",
                                        name=f"et_{hp}_{j}_{p}_{he}")
                        if d0 == 0:
                            nc.scalar.activation(et[:, :], sc[:, :], Exp,
                                                 scale=float(SCALE))
                        else:
                            et3 = et[:, :].rearrange("p (g q) -> p g q", g=2)
                            sc3 = sc[:, :].rearrange("p (g q) -> p g q", g=2)
                            nc.scalar.activation(et3[:, :, d0:512],
                                                 sc3[:, :, d0:512], Exp,
                                                 scale=float(SCALE))
                        est["pe"] += 2 * (512 - d0) * 0.42
                        est["act"] += 2 * (512 - d0) * 0.833 + 250
                        # mask the 128x128 diagonal blocks
                        if q0e >= 0:
                            sl = slice(q0e * 128, (q0e + 1) * 128)
                            nc.vector.tensor_mul(et[:, sl], et[:, sl], tri[:, :])
                        if 0 <= q0o < 4:
                            sl = slice(512 + q0o * 128, 512 + (q0o + 1) * 128)
                            nc.vector.tensor_mul(et[:, sl], et[:, sl], tri[:, :])
                        ets.append(et)
                    etq.append((p, ets))
                    if len(etq) > 1:
                        emit_av(*etq.popleft())
                    fill(TAIL_FILL_NS if in_tail[0] else None)
                while etq:
                    fill(TAIL_FILL_NS if in_tail[0] else None)
                    emit_av(*etq.popleft())
                # evictions: all reads must follow the bank's group stop
                rcps = []
                for he in (0, 1):
                    rcp = spool.tile([128, 4], f32, tag="rcp",
                                     name=f"rcp_{hp}_{j}_{he}")
                    nc.vector.reciprocal(
                        rcp[:, :],
                        yu[he][:, :].rearrange("p (q e) -> p q e", q=4)[:, :, 64])
                    rcps.append(rcp)
                for q in range(4):
                    t = 4 * j + q
                    ybn = spool.tile([128, 128], bf16, tag="ybn",
                                     name=f"ybn_{hp}_{t}")
                    for he in (0, 1):
                        nc.vector.tensor_scalar_mul(
                            ybn[:, he * 64:(he + 1) * 64],
                            yu[he][:, q * 65: q * 65 + 64],
                            rcps[he][:, q:q + 1])
                    nc.sync.dma_start_transpose(
                        yT[:, fb + t * 128: fb + (t + 1) * 128], ybn[:, :])
                    if hp == 1:
                        if j == TAIL_J:
                            fillq.append(make_proj(t, tail=True))
                        else:
                            projq.append(t)

            # ---- schedule ----
            # prefix: q/k over tq [0:512); q_dc1 bridges the wk DMA latency,
            # k_dc1(tb0) goes to the fill queue so attention starts sooner.
            for lbl, cost, fn in qk_parts(wq, bq, qT, 0, 0, "qdc0tb0"):
                fn()
            for lbl, cost, fn in qk_parts(wq, bq, qT, 1, 0, "qdc1tb0"):
                fn()
            for lbl, cost, fn in qk_parts(wk, bk, kT, 0, 0, "kdc0tb0"):
                fn()
            done.add("qdc1tb0p2")

            # fill queue holds ALL non-attention PE work, spread across the
            # attention pair-steps; drain_until guards force what a block
            # needs before it runs.
            fillq.extend(qk_parts(wk, bk, kT, 1, 0, "kdc1tb0"))
            for tt in range(4):
                fillq.extend(v_parts(tt))
            fillq.extend(qk_parts(wq, bq, qT, 0, 1, "qdc0tb1"))
            fillq.extend(qk_parts(wk, bk, kT, 0, 1, "kdc0tb1"))
            for tt in range(4, 8):
                fillq.extend(v_parts(tt))
            fillq.extend(qk_parts(wq, bq, qT, 0, 2, "qdc0tb2"))
            fillq.extend(qk_parts(wk, bk, kT, 0, 2, "kdc0tb2"))
            for tt in range(8, 12):
                fillq.extend(v_parts(tt))
            fillq.extend(qk_parts(wq, bq, qT, 0, 3, "qdc0tb3"))
            fillq.extend(qk_parts(wk, bk, kT, 0, 3, "kdc0tb3"))
            for tt in range(12, 16):
                fillq.extend(v_parts(tt))
            fillq.extend(qk_parts(wq, bq, qT, 1, 1, "qdc1tb1"))
            fillq.extend(qk_parts(wk, bk, kT, 1, 1, "kdc1tb1"))
            fillq.extend(qk_parts(wq, bq, qT, 1, 2, "qdc1tb2"))
            fillq.extend(qk_parts(wk, bk, kT, 1, 2, "kdc1tb2"))
            fillq.extend(qk_parts(wq, bq, qT, 1, 3, "qdc1tb3"))
            fillq.extend(qk_parts(wk, bk, kT, 1, 3, "kdc1tb3"))

            attn(0, 0)
            for j in (1, 2, 3):
                drain_until(f"qdc0tb{j}p2")
                attn(0, j,
                     prediag=(lambda jj=j: drain_until(f"kdc0tb{jj}p2")))

            # hp1 in order [1, 2, 3, 0]: each block's proj tiles become the
            # next block's filler, and the tail block (j0) is the shortest
            # dependency chain with j3's proj tiles as its filler.
            for j in (1, 2, 3, 0):
                drain_until(f"qdc1tb{j}p2")
                in_tail[0] = (j == TAIL_J)
                attn(1, j,
                     prediag=(lambda jj=j: drain_until(f"kdc1tb{jj}p2")))

            # tail: leftover proj tiles drain with Act-assisted copies (no
            # exps remain for them to delay), then any remaining filler
            while projq:
                _, _, fn = make_proj(projq.popleft(), tail=True)
                fn()
            while fillq:
                lbl, cost, fn = fillq.popleft()
                fn()
                done.add(lbl)

    nc.compile()
    return nc


def get_program():
    if "nc" not in _CACHE:
        _CACHE["nc"] = _build_program()
    return _CACHE["nc"]


def _pack_cmajor(a):
    """[C_rows, N] -> [128, (C_rows/128)*N] with chunk c at [:, c*N:(c+1)*N]."""
    rows, n = a.shape
    return np.ascontiguousarray(
        a.reshape(rows // 128, 128, n).transpose(1, 0, 2).reshape(128, -1))


def make_in_maps(x, W_attn, b_attn, W_proj):
    """Host-side sharding: per-core input dict."""
    x = np.asarray(x, np.float32)
    W_attn = np.asarray(W_attn, np.float32)
    b_attn = np.asarray(b_attn, np.float32)
    W_proj = np.asarray(W_proj, np.float32)

    tk = np.arange(128)[:, None]
    tq = np.arange(128)[None, :]
    tri = (tq >= tk).astype(BF16)
    ident = np.eye(128, dtype=BF16)

    xT_b = [_pack_cmajor(x[b].T.astype(BF16)) for b in range(B)]

    in_maps = []
    for g in range(N_CORES):
        b, hg = divmod(g, 4)
        cs = slice(hg * DL, (hg + 1) * DL)
        wq = _pack_cmajor(W_attn[:, 0 * C:1 * C][:, cs].astype(BF16))
        wk = _pack_cmajor(W_attn[:, 1 * C:2 * C][:, cs].astype(BF16))
        wv = _pack_cmajor(W_attn[:, 2 * C:3 * C][:, cs].astype(BF16))
        wp = _pack_cmajor(W_proj[cs, :].astype(BF16))
        bq = np.ascontiguousarray(b_attn[0 * C:1 * C][cs].reshape(2, 128).T)
        bk = np.ascontiguousarray(b_attn[1 * C:2 * C][cs].reshape(2, 128).T)
        bvr = np.ascontiguousarray(np.tile(b_attn[2 * C:3 * C][cs][None, :], (128, 1)))
        in_maps.append({
            "xTp": xT_b[b],
            "wqp": wq, "wkp": wk, "wvp": wv, "wpp": wp,
            "bq": bq.astype(np.float32), "bk": bk.astype(np.float32),
            "bvr": bvr.astype(np.float32),
            "tri": tri, "ident": ident,
        })
    return in_maps


def assemble_output(results, b_proj):
    """results: per-core dicts with 'out' [T, C] partials."""
    b_proj = np.asarray(b_proj, np.float32)
    out = np.zeros((B, T, C), np.float32)
    for g in range(N_CORES):
        out[g // 4] += np.asarray(results[g]["out"], np.float32)
    out += b_proj[None, None, :]
    return out


def kernel(x, W_attn, b_attn, W_proj, b_proj):
    from concourse.bass_utils import run_bass_kernel_spmd

    nc = get_program()
    in_maps = make_in_maps(x, W_attn, b_attn, W_proj)
    res = run_bass_kernel_spmd(nc, in_maps, list(range(N_CORES)))
    return assemble_output(res.results, b_proj)
